# revision 3
# baseline (speedup 1.0000x reference)
"""HaarPool GNN (GIN + Haar pooling) Trainium2 kernel, 8-core data-parallel.

Strategy:
- Shard the 1024-graph batch across 8 cores (128 graphs / 16384 nodes each).
- Message passing segment_sum is recast as dense per-graph adjacency matmuls:
  host builds B'_g[src,dst] = multi-edge count + I (the +I folds `x + agg`).
  Layers 1/2 pack 2/4 pooled graphs block-diagonally into 128x128 tiles.
- Activations live transposed in SBUF: [H=128 partitions, nodes free].
  MLP matmuls keep weights stationary (lhsT=W), streaming x^T.
  Per-graph PE transposes produce node-major tiles for the agg matmul lhsT.
- Training-mode BatchNorm has global batch stats -> per-core [mean, E[x^2]]
  partials are AllGather'd (8 ranks, ~5us floor) and re-reduced locally.
  BN apply + ReLU is one fused ScalarE activation pass (per-partition
  scale/bias). All pre-BN biases cancel exactly in training-mode BN.
- Haar pool = strided pair-add (DVE); its 1/sqrt(2) is folded into the BN
  scale (ReLU is positively homogeneous). Global add pool = segmented
  free-dim reduce. The 3 embedding BNs are deferred into one final AllGather.
"""
import numpy as np

N = 131072
G = 1024
NPG = 128
H = 128
L = 3
C = 10
BN_EPS = 1e-5
INV_SQRT2 = 0.7071067811865476
N_CORES = 8
GC = G // N_CORES          # graphs per core = 128
NC0 = N // N_CORES         # nodes per core  = 16384

_CACHE = {}


def _build_nc():
    import concourse.bass as bass
    import concourse.bacc as bacc
    import concourse.mybir as mybir
    import concourse.tile as tile

    fp32 = mybir.dt.float32
    AF = mybir.ActivationFunctionType
    ALU = mybir.AluOpType
    AX = mybir.AxisListType

    nc = bacc.Bacc("TRN2", target_bir_lowering=False, debug=False,
                   enable_asserts=False, num_devices=N_CORES)

    # ---- DRAM I/O ----
    d_xt = nc.dram_tensor("xt", [H, NC0], fp32, kind="ExternalInput").ap()
    d_b0 = nc.dram_tensor("b0", [GC * 128, 128], fp32, kind="ExternalInput").ap()
    d_b1 = nc.dram_tensor("b1", [(GC // 2) * 128, 128], fp32, kind="ExternalInput").ap()
    d_b2 = nc.dram_tensor("b2", [(GC // 4) * 128, 128], fp32, kind="ExternalInput").ap()
    d_ws = nc.dram_tensor("ws", [H, H], fp32, kind="ExternalInput").ap()
    d_w1 = nc.dram_tensor("w1", [L * H, H], fp32, kind="ExternalInput").ap()
    d_w2 = nc.dram_tensor("w2", [L * H, H], fp32, kind="ExternalInput").ap()
    d_bnp = nc.dram_tensor("bnp", [H, 20], fp32, kind="ExternalInput").ap()
    d_lw = nc.dram_tensor("lw", [L * H, C], fp32, kind="ExternalInput").ap()
    d_lb = nc.dram_tensor("lb", [C, 1], fp32, kind="ExternalInput").ap()
    d_id = nc.dram_tensor("ident", [H, H], fp32, kind="ExternalInput").ap()
    d_out = nc.dram_tensor("out_t", [C, GC], fp32, kind="ExternalOutput").ap()

    with tile.TileContext(nc) as tc:
        with tc.tile_pool(name="const", bufs=1) as cpool, \
             tc.tile_pool(name="big", bufs=1) as big, \
             tc.tile_pool(name="bt", bufs=4) as btp, \
             tc.tile_pool(name="misc", bufs=1) as misc, \
             tc.tile_pool(name="psA", bufs=2, space="PSUM") as psA, \
             tc.tile_pool(name="psM", bufs=3, space="PSUM") as psM, \
             tc.tile_pool(name="psH", bufs=1, space="PSUM") as psH, \
             tc.tile_pool(name="dram", bufs=1, space="DRAM") as dram:

            # ---- constants into SBUF ----
            ident = cpool.tile([H, H], fp32, tag="ident")
            nc.sync.dma_start(ident[:], d_id[:])
            ws = cpool.tile([H, H], fp32, tag="ws")
            nc.sync.dma_start(ws[:], d_ws[:])
            w1 = cpool.tile([H, L * H], fp32, tag="w1")
            w2 = cpool.tile([H, L * H], fp32, tag="w2")
            for l in range(L):
                nc.sync.dma_start(w1[:, l * H:(l + 1) * H], d_w1[l * H:(l + 1) * H, :])
                nc.sync.dma_start(w2[:, l * H:(l + 1) * H], d_w2[l * H:(l + 1) * H, :])
            bnp = cpool.tile([H, 20], fp32, tag="bnp")
            nc.sync.dma_start(bnp[:], d_bnp[:])
            zeros = cpool.tile([H, 1], fp32, tag="zeros")
            nc.vector.memset(zeros[:], 0.0)
            nc.const_aps.aps[(fp32, 0.0)] = zeros[:]
            lw = cpool.tile([H, L * C], fp32, tag="lw")
            for l in range(L):
                nc.sync.dma_start(lw[:, l * C:(l + 1) * C], d_lw[l * H:(l + 1) * H, :])
            lb = cpool.tile([C, 1], fp32, tag="lb")
            nc.sync.dma_start(lb[:], d_lb[:])
            embds = cpool.tile([H, L * H], fp32, tag="embds")
            embp = cpool.tile([H, L * H], fp32, tag="embp")
            stats_e = cpool.tile([H, L * 6], fp32, tag="stats_e")

            ag_idx = [0]

            def bn_sync(payload_cols):
                """AllGather a [128, payload_cols] partial-stat tile across the
                8 cores; return a [128, payload_cols] tile of rank-summed stats
                (caller divides by 8). payload tile returned for writing."""
                i = ag_idx[0]
                ag_idx[0] += 1
                pay = misc.tile([H, payload_cols], fp32, tag=f"pay{i}")
                cin = dram.tile([H, payload_cols], fp32, tag=f"agin{i}")
                cout = dram.tile([H * N_CORES, payload_cols], fp32,
                                 addr_space="Shared", tag=f"agout{i}")
                back = misc.tile([H, payload_cols * N_CORES], fp32, tag=f"back{i}")
                glob = misc.tile([H, payload_cols], fp32, tag=f"glob{i}")

                def fire():
                    nc.sync.dma_start(cin[:], pay[:])
                    nc.gpsimd.collective_compute(
                        "AllGather", ALU.bypass,
                        ins=[cin[:]], outs=[cout[:]],
                        replica_groups=[list(range(N_CORES))],
                    )
                    # back[p, c, r] <- cout[r*128+p, c]
                    nc.sync.dma_start(
                        back[:].rearrange("p (c r) -> p c r", r=N_CORES),
                        cout[:].rearrange("(r p) c -> p c r", r=N_CORES),
                    )
                    nc.vector.tensor_reduce(
                        glob[:],
                        back[:].rearrange("p (c r) -> p c r", r=N_CORES),
                        axis=AX.X, op=ALU.add)
                return pay, glob, fire

            def stats_to_st(glob, gcol, bcol, pool_scale, i):
                """glob = [sum over ranks of (mean, E2)] -> s,t vectors."""
                mv8 = misc.tile([H, 2], fp32, tag=f"mv8_{i}")
                nc.vector.tensor_scalar(mv8[:], glob[:, 0:2], 1.0 / N_CORES, None,
                                        op0=ALU.mult)
                mg2 = misc.tile([H, 1], fp32, tag=f"mg2_{i}")
                nc.vector.tensor_mul(mg2[:], mv8[:, 0:1], mv8[:, 0:1])
                vg = misc.tile([H, 1], fp32, tag=f"vg_{i}")
                # var + eps = (E2 - mean^2) + eps in one tensor_scalar
                nc.vector.tensor_scalar(vg[:], mv8[:, 1:2], mg2[:], BN_EPS,
                                        op0=ALU.subtract, op1=ALU.add)
                sd = misc.tile([H, 1], fp32, tag=f"sd_{i}")
                nc.scalar.activation(sd[:], vg[:], AF.Sqrt, bias=0.0, scale=1.0)
                rsd = misc.tile([H, 1], fp32, tag=f"rsd_{i}")
                nc.vector.reciprocal(rsd[:], sd[:])
                svec = misc.tile([H, 1], fp32, tag=f"svec_{i}")
                nc.vector.tensor_mul(svec[:], rsd[:], bnp[:, gcol:gcol + 1])
                ms = misc.tile([H, 1], fp32, tag=f"ms_{i}")
                nc.vector.tensor_mul(ms[:], mv8[:, 0:1], svec[:])
                tvec = misc.tile([H, 1], fp32, tag=f"tvec_{i}")
                nc.vector.tensor_sub(tvec[:], bnp[:, bcol:bcol + 1], ms[:])
                if pool_scale:
                    nc.vector.tensor_scalar(svec[:], svec[:], INV_SQRT2, None,
                                            op0=ALU.mult)
                    nc.vector.tensor_scalar(tvec[:], tvec[:], INV_SQRT2, None,
                                            op0=ALU.mult)
                return svec, tvec

            def mlp_bn(src_big, wtile, wcol, n, raw_tag, gcol, bcol, pool_scale):
                """raw = W^T @ src (chunks of 512), bn_stats per chunk,
                AllGather stats, return (raw, svec, tvec)."""
                nch = n // 512
                raw = big.tile([H, n], fp32, tag=raw_tag)
                st = misc.tile([H, 6 * nch], fp32, tag="stats")
                for c in range(nch):
                    pm = psM.tile([H, 512], fp32, tag="pm")
                    nc.tensor.matmul(pm[:], wtile[:, wcol:wcol + H],
                                     src_big[:, c * 512:(c + 1) * 512],
                                     start=True, stop=True)
                    nc.vector.bn_stats(st[:, c * 6:(c + 1) * 6], pm[:])
                    nc.scalar.copy(raw[:, c * 512:(c + 1) * 512], pm[:])
                mv = misc.tile([H, 2], fp32, tag="mv")
                nc.vector.bn_aggr(mv[:], st[:])
                i = ag_idx[0]
                pay, glob, fire = bn_sync(2)
                nc.vector.tensor_copy(pay[:, 0:1], mv[:, 0:1])
                # E2 = var + mean^2
                m2 = misc.tile([H, 1], fp32, tag=f"m2_{i}")
                nc.vector.tensor_mul(m2[:], mv[:, 0:1], mv[:, 0:1])
                nc.vector.tensor_add(pay[:, 1:2], mv[:, 1:2], m2[:])
                fire()
                svec, tvec = stats_to_st(glob, gcol, bcol, pool_scale, i)
                return raw, svec, tvec

            # ================= stage 0: lin_start + BN + relu =================
            xa = big.tile([H, NC0 // 2], fp32, tag="A")
            xb = big.tile([H, NC0 // 2], fp32, tag="B")
            nc.sync.dma_start(xa[:], d_xt[:, 0:NC0 // 2])
            nc.sync.dma_start(xb[:], d_xt[:, NC0 // 2:NC0])
            nch = NC0 // 512
            raw0 = big.tile([H, NC0], fp32, tag="R")
            st0 = misc.tile([H, 6 * nch], fp32, tag="stats")
            for c in range(nch):
                src = xa if c < nch // 2 else xb
                off = c * 512 if c < nch // 2 else (c - nch // 2) * 512
                pm = psM.tile([H, 512], fp32, tag="pm")
                nc.tensor.matmul(pm[:], ws[:], src[:, off:off + 512],
                                 start=True, stop=True)
                nc.vector.bn_stats(st0[:, c * 6:(c + 1) * 6], pm[:])
                nc.scalar.copy(raw0[:, c * 512:(c + 1) * 512], pm[:])
            mv0 = misc.tile([H, 2], fp32, tag="mv")
            nc.vector.bn_aggr(mv0[:], st0[:])
            pay, glob, fire = bn_sync(2)
            nc.vector.tensor_copy(pay[:, 0:1], mv0[:, 0:1])
            m20 = misc.tile([H, 1], fp32, tag="m20")
            nc.vector.tensor_mul(m20[:], mv0[:, 0:1], mv0[:, 0:1])
            nc.vector.tensor_add(pay[:, 1:2], mv0[:, 1:2], m20[:])
            fire()
            sv, tv = stats_to_st(glob, 0, 1, False, "s0")
            x_cur = big.tile([H, NC0], fp32, tag="A")
            nc.scalar.activation(x_cur[:], raw0[:], AF.Relu,
                                 bias=tv[:, 0:1], scale=sv[:, 0:1])
            # rotation state: x_cur in tag A; tags B and R free
            tags = {"x": "A", "free1": "B", "free2": "R"}

            # ================= GNN layers =================
            d_bs = [d_b0, d_b1, d_b2]
            n = NC0
            for l in range(L):
                d_b = d_bs[l]
                nwin = n // 128          # transpose windows / B tiles
                # --- transposes: x^T -> node-major windows ---
                xn = big.tile([H, n], fp32, tag=tags["free1"])
                for w4 in range(nwin // 4):
                    pt = psA.tile([H, 512], fp32, tag="pa")
                    for j in range(4):
                        w = w4 * 4 + j
                        nc.tensor.transpose(pt[:, j * 128:(j + 1) * 128],
                                            x_cur[:, w * 128:(w + 1) * 128],
                                            ident[:])
                    nc.scalar.copy(xn[:, w4 * 512:(w4 + 1) * 512], pt[:])
                # --- agg matmuls: h_in^T = (B'_g)^T-contracted windows ---
                hin = big.tile([H, n], fp32, tag=tags["free2"])
                for w4 in range(nwin // 4):
                    pa = psA.tile([H, 512], fp32, tag="pa")
                    for j in range(4):
                        w = w4 * 4 + j
                        bt = btp.tile([128, 128], fp32, tag="bt")
                        nc.sync.dma_start(bt[:], d_b[w * 128:(w + 1) * 128, :])
                        nc.tensor.matmul(pa[:, j * 128:(j + 1) * 128],
                                         xn[:, w * 128:(w + 1) * 128], bt[:],
                                         start=True, stop=True)
                    nc.scalar.copy(hin[:, w4 * 512:(w4 + 1) * 512], pa[:])
                tags = {"x": tags["free2"], "free1": tags["x"], "free2": tags["free1"]}
                # now: hin in tags["x"]; free1 = old x tag; free2 = xn tag (xn
                # dead after agg)

                # --- MLP1 + BN + relu ---
                raw1, sv1, tv1 = mlp_bn(hin, w1, l * H, n, tags["free1"],
                                        2 + l, 5 + l, False)
                h1 = big.tile([H, n], fp32, tag=tags["free2"])
                nc.scalar.activation(h1[:], raw1[:], AF.Relu,
                                     bias=tv1[:, 0:1], scale=sv1[:, 0:1])
                tags = {"x": tags["free2"], "free1": tags["x"], "free2": tags["free1"]}
                # h1 live; hin dead; raw1 dead after activation

                # --- MLP2 + BN + relu (pool 1/sqrt2 folded into s,t) ---
                raw2, sv2, tv2 = mlp_bn(h1, w2, l * H, n, tags["free1"],
                                        8 + l, 11 + l, True)
                xs = big.tile([H, n], fp32, tag=tags["free2"])
                nc.scalar.activation(xs[:], raw2[:], AF.Relu,
                                     bias=tv2[:, 0:1], scale=sv2[:, 0:1])
                tags = {"x": tags["free2"], "free1": tags["x"], "free2": tags["free1"]}

                # --- Haar pool: pair-add (scale already folded) ---
                n2 = n // 2
                xp = big.tile([H, n2], fp32, tag=tags["free2"])
                xse = xs[:].rearrange("p (k two) -> p k two", two=2)
                nc.vector.tensor_add(xp[:], xse[:, :, 0], xse[:, :, 1])
                tags = {"x": tags["free2"], "free1": tags["x"], "free2": tags["free1"]}

                # --- global add pool -> embd_l, deferred BN stats ---
                npg2 = n2 // GC
                nc.vector.tensor_reduce(
                    embds[:, l * H:(l + 1) * H],
                    xp[:].rearrange("p (g m) -> p g m", m=npg2),
                    axis=AX.X, op=ALU.add)
                nc.vector.bn_stats(stats_e[:, l * 6:(l + 1) * 6],
                                   embds[:, l * H:(l + 1) * H])
                x_cur = xp
                n = n2

            # ================= embedding BNs (one AllGather) + head ===========
            pay, glob, fire = bn_sync(2 * L)
            for l in range(L):
                mve = misc.tile([H, 2], fp32, tag=f"mve{l}")
                nc.vector.bn_aggr(mve[:], stats_e[:, l * 6:(l + 1) * 6])
                nc.vector.tensor_copy(pay[:, 2 * l:2 * l + 1], mve[:, 0:1])
                m2e = misc.tile([H, 1], fp32, tag=f"m2e{l}")
                nc.vector.tensor_mul(m2e[:], mve[:, 0:1], mve[:, 0:1])
                nc.vector.tensor_add(pay[:, 2 * l + 1:2 * l + 2], mve[:, 1:2], m2e[:])
            fire()
            ph = psH.tile([C, GC], fp32, tag="ph")
            for l in range(L):
                gl = misc.tile([H, 2], fp32, tag=f"globe{l}")
                nc.vector.tensor_copy(gl[:], glob[:, 2 * l:2 * l + 2])
                sve, tve = stats_to_st(gl, 14 + l, 17 + l, False, f"e{l}")
                nc.scalar.activation(embp[:, l * H:(l + 1) * H],
                                     embds[:, l * H:(l + 1) * H], AF.Relu,
                                     bias=tve[:, 0:1], scale=sve[:, 0:1])
                nc.tensor.matmul(ph[:], lw[:, l * C:(l + 1) * C],
                                 embp[:, l * H:(l + 1) * H],
                                 start=(l == 0), stop=(l == L - 1))
            out_sb = misc.tile([C, GC], fp32, tag="out_sb")
            nc.scalar.activation(out_sb[:], ph[:], AF.Identity,
                                 bias=lb[:, 0:1], scale=1.0)
            nc.sync.dma_start(d_out[:], out_sb[:])

    nc.compile()
    return nc


def _build_in_maps(inputs):
    x = np.asarray(inputs["x"], dtype=np.float32)
    ei = np.asarray(inputs["edge_index"])
    src = ei[0].astype(np.int64)
    dst = ei[1].astype(np.int64)
    g = src // NPG
    sl = src % NPG
    dl = dst % NPG

    def adj(shift, wing):
        """wing graphs packed block-diagonally per 128x128 window."""
        npg = NPG >> shift
        win = g // wing
        sub = g % wing
        row = sub * npg + (sl >> shift)
        col = sub * npg + (dl >> shift)
        idx = win * (128 * 128) + row * 128 + col
        nwin = G // wing
        B = np.bincount(idx, minlength=nwin * 128 * 128).astype(np.float32)
        B = B.reshape(nwin, 128, 128)
        B += np.eye(128, dtype=np.float32)[None]
        return B

    B0 = adj(0, 1)
    B1 = adj(1, 2)
    B2 = adj(2, 4)

    bnp = np.zeros((H, 20), dtype=np.float32)
    bnp[:, 0] = np.asarray(inputs["bn_start_g"], np.float32)
    bnp[:, 1] = np.asarray(inputs["bn_start_b"], np.float32)
    for l in range(L):
        bnp[:, 2 + l] = np.asarray(inputs["conv_bn_g"], np.float32)[l]
        bnp[:, 5 + l] = np.asarray(inputs["conv_bn_b"], np.float32)[l]
        bnp[:, 8 + l] = np.asarray(inputs["bn_g"], np.float32)[l]
        bnp[:, 11 + l] = np.asarray(inputs["bn_b"], np.float32)[l]
        bnp[:, 14 + l] = np.asarray(inputs["bne_g"], np.float32)[l]
        bnp[:, 17 + l] = np.asarray(inputs["bne_b"], np.float32)[l]

    shared = dict(
        ws=np.ascontiguousarray(np.asarray(inputs["lin_start_w"], np.float32)),
        w1=np.ascontiguousarray(
            np.asarray(inputs["conv_w1"], np.float32).reshape(L * H, H)),
        w2=np.ascontiguousarray(
            np.asarray(inputs["conv_w2"], np.float32).reshape(L * H, H)),
        bnp=bnp,
        lw=np.ascontiguousarray(np.asarray(inputs["lin_w"], np.float32)),
        lb=np.asarray(inputs["lin_b"], np.float32).reshape(C, 1).copy(),
        ident=np.eye(H, dtype=np.float32),
    )
    in_maps = []
    for c in range(N_CORES):
        m = dict(shared)
        m["xt"] = np.ascontiguousarray(x[c * NC0:(c + 1) * NC0].T)
        m["b0"] = np.ascontiguousarray(
            B0[c * GC:(c + 1) * GC].reshape(GC * 128, 128))
        m["b1"] = np.ascontiguousarray(
            B1[c * (GC // 2):(c + 1) * (GC // 2)].reshape((GC // 2) * 128, 128))
        m["b2"] = np.ascontiguousarray(
            B2[c * (GC // 4):(c + 1) * (GC // 4)].reshape((GC // 4) * 128, 128))
        in_maps.append(m)
    return in_maps


def _run(inputs, trace=False, tmpdir=None):
    from concourse import bass_utils
    if "nc" not in _CACHE:
        _CACHE["nc"] = _build_nc()
    nc = _CACHE["nc"]
    in_maps = _build_in_maps(inputs)
    res = bass_utils.run_bass_kernel_spmd(
        nc, in_maps, core_ids=list(range(N_CORES)),
        trace=trace, tmpdir=tmpdir)
    out = np.concatenate(
        [res.results[c]["out_t"].T for c in range(N_CORES)], axis=0)
    return out.astype(np.float32), res


def kernel(**inputs) -> np.ndarray:
    out, _ = _run(inputs)
    return out
